# revision 16
# baseline (speedup 1.0000x reference)
"""Trainium2 Bass kernel for nn_DynamicHead — contiguous sharding + linear basis.

Within a knot segment the function is an exact cubic in t.  Sort all samples
by t, give each core a contiguous range of 4096, and split each core's range
into knot-pure chunks of <= 512 samples.  Each chunk spans a t-width of only
~0.016, so after recentering at the chunk midpoint a LINEAR basis [1, dt]
suffices: the quadratic/cubic terms are folded minimax-style (Chebyshev)
into [1, dt] with relative residual ~2e-4 per layer.  K = 2*256 = 512 per
layer (vs 1024 exact), and each core only needs weight tables for its own
~9 chunks (~2.5 MB vs 21 MB round-robin fp32).

All matmul/DVE operands bf16 (fp32 PSUM).  Device per chunk: z1 = x*dt via
DVE (dt broadcast per chunk via DMA), per half 4 k-tile matmuls + K=2 bias
matmul, ACT relu-evac; head contracts to [2,cap], bias via DVE per-partition
add, *[1;dt] then ones-matmul.  Chunk-skewed pipeline + HAM warmup matmuls.
"""
import os
import sys
import types

for _p in ('/opt/trn_rl_repo', '/root/.axon_site/_ro/trn_rl_repo'):
    if _p not in sys.path:
        sys.path.append(_p)

import numpy as np
import ml_dtypes
import concourse.bass as bass
import concourse.tile as tile
from concourse import bacc, mybir
from concourse import bass_utils

F32 = mybir.dt.float32
BF16 = mybir.dt.bfloat16
NPBF = ml_dtypes.bfloat16
RELU = mybir.ActivationFunctionType.Relu
COPY = mybir.ActivationFunctionType.Copy
IDENT = mybir.ActivationFunctionType.Identity

B, D, NSEG = 32768, 256, 9
NP = 2                                  # linear centered basis [1, dt]
KT = NP * D // 128                      # 4 k-tiles of 128
CAPMAX = 512
N_CORES = 8
BPC = B // N_CORES
KNOTS = np.array([i / 9.0 for i in range(1, 9)], dtype=np.float64)
SDIM = 12

TRACE = False
LAST_EXEC_NS = None
LAST_MEAN_EXEC_NS = None
LAST_RES = None

_PROG_CACHE = {}

if os.environ.get("BASS_LDW_OPT") == "1":
    _orig_run_command = bass_utils.run_command

    def _run_command_ldw(argv, **kw):
        argv = ["--enable-ldw-opt=true" if a == "--enable-ldw-opt=false" else a
                for a in argv]
        return _orig_run_command(argv, **kw)

    bass_utils.run_command = _run_command_ldw


def _register_ntff_hook():
    try:
        import antenv.axon_hooks  # noqa: F401
        return
    except ImportError:
        pass
    try:
        from trn_agent_boot.trn_boot import _ntff_profile_via_ctypes
        hook = _ntff_profile_via_ctypes('/opt/axon/libaxon_pjrt.so')
        mod = types.ModuleType('antenv.axon_hooks')
        mod.get_axon_ntff_profile_hook = lambda: hook
        sys.modules['antenv.axon_hooks'] = mod
    except Exception:
        pass


def _gamma4() -> np.ndarray:
    """(NSEG, 4, SDIM): basis -> per-segment cubic coefficients (t-basis)."""
    g = np.zeros((NSEG, 4, SDIM), dtype=np.float64)
    for m in range(NSEG):
        for p in range(4):
            g[m, p, p] = 1.0
        for j in range(1, 9):          # spline s = 3 + j, knot k = j/9
            if j <= m:
                k = KNOTS[j - 1]
                g[m, 0, 3 + j] = -k ** 3
                g[m, 1, 3 + j] = 3 * k ** 2
                g[m, 2, 3 + j] = -3 * k
                g[m, 3, 3 + j] = 1.0
    return g


def _relin(c4, t0, h):
    """cubic coeffs (4, ...) in t-basis -> linear (2, ...) in dt-basis.

    Taylor recenter at t0, then Chebyshev minimax folds on [-h, h]:
    dt^2 ~ h^2/2 (into const), dt^3 ~ (3h^2/4) dt (into linear)."""
    from math import comb
    c = np.zeros((4,) + c4.shape[1:])
    for q in range(4):
        for p in range(q, 4):
            c[q] += comb(p, q) * (t0 ** (p - q)) * c4[p]
    out = c[:2].copy()
    out[0] += 0.5 * h * h * c[2]
    out[1] += 0.75 * h * h * c[3]
    return out


def _build_program(caps):
    """SPMD single-core program: NSLOT chunks with per-slot capacities."""
    caps = tuple(int(c) for c in caps)
    nslot = len(caps)
    offs = [0]
    for c in caps:
        offs.append(offs[-1] + c)
    bp = offs[-1]
    nc = bacc.Bacc("TRN2", target_bir_lowering=False, debug=False,
                   num_devices=N_CORES)

    xT_ap = nc.dram_tensor("xT", [128, 2 * bp], BF16, kind="ExternalInput").ap()
    tp_ap = nc.dram_tensor("tp", [NP, bp], BF16, kind="ExternalInput").ap()
    c0w_ap = nc.dram_tensor("c0w", [nslot, 128, KT * D], BF16, kind="ExternalInput").ap()
    c1w_ap = nc.dram_tensor("c1w", [nslot, 128, KT * D], BF16, kind="ExternalInput").ap()
    cbw_ap = nc.dram_tensor("cbw", [NP, 2 * nslot * D], BF16, kind="ExternalInput").ap()
    c2_ap = nc.dram_tensor("c2", [128, nslot * 2 * NP], BF16, kind="ExternalInput").ap()
    c2b_ap = nc.dram_tensor("c2b", [NP, nslot], F32, kind="ExternalInput").ap()
    ones_ap = nc.dram_tensor("ones2", [NP, 1], BF16, kind="ExternalInput").ap()
    out_ap = nc.dram_tensor("out", [1, bp], F32, kind="ExternalOutput").ap()

    cw_ap = (c0w_ap, c1w_ap)

    with tile.TileContext(nc) as tc:
        with (
            tc.tile_pool(name="act", bufs=1) as actp,
            tc.tile_pool(name="bc", bufs=1) as bcp,
            tc.tile_pool(name="z", bufs=1) as zp,
            tc.tile_pool(name="w", bufs=1) as wp,
            tc.tile_pool(name="sm", bufs=1) as smp,
            tc.tile_pool(name="pm", bufs=1, space="PSUM") as pmp,
            tc.tile_pool(name="pq", bufs=1, space="PSUM") as pqp,
        ):
            # ---- HAM warmup: keep PE busy through the DMA prologue so the
            # clock gate opens before real work arrives.
            wu = smp.tile([128, 512], BF16, name="wu", tag="wu")
            nc.vector.memset(wu[:, :], 0)
            pwu = pqp.tile([128, 512], F32, name="pwu", tag="pq", bufs=1)
            for _ in range(7):
                nc.tensor.matmul(pwu[:, :], wu[:, 0:128], wu[:, :],
                                 start=True, stop=True)

            wts = {}

            def wload(L, s):
                # layer 0 weights on the sync ring, layer 1 on the vector
                # ring: two HWDGE rings in parallel so early weight supply
                # keeps up with PE consumption (one ring serializes at
                # ~2.3us per tile, about the PE's per-slot-layer rate)
                wt = wp.tile([128, KT * D], BF16, name=f"w{L}_{s}",
                             tag=f"w{L}", bufs=4)
                eng = nc.sync if L == 0 else nc.gpsimd
                eng.dma_start(wt[:, :], cw_ap[L][s])
                wts[(L, s)] = wt

            # first two L0 weight tiles lead the sync ring so the early
            # matmuls aren't stuck behind the small one-time loads (each
            # ring entry costs ~2us of completion-serialized latency)
            wload(0, 0)
            wload(0, 1)
            wload(0, 2)

            # ---- one-time loads ----
            tps = smp.tile([NP, bp], BF16, name="tps", tag="tps")
            nc.sync.dma_start(tps[:, :], tp_ap[:, :])
            cbw = smp.tile([NP, 2 * nslot * D], BF16, name="cbw", tag="cbw")
            nc.sync.dma_start(cbw[:, :], cbw_ap[:, :])
            c2t = smp.tile([128, nslot * 2 * NP], BF16, name="c2t", tag="c2t")
            nc.sync.dma_start(c2t[:, :], c2_ap[:, :])
            c2b = smp.tile([NP, nslot], F32, name="c2b", tag="c2b")
            nc.sync.dma_start(c2b[:, :], c2b_ap[:, :])
            ones2 = smp.tile([NP, 1], BF16, name="ones2", tag="ones2")
            nc.sync.dma_start(ones2[:, :], ones_ap[:, :])
            out_all = smp.tile([1, bp], F32, name="out_all", tag="out_all")

            xin, x1, x2, t1s = {}, {}, {}, {}

            def load_seg(s):
                cap, off = caps[s], offs[s]
                xt = actp.tile([128, 2 * cap], BF16, name=f"xin{s}",
                               tag="xin", bufs=4)
                nc.scalar.dma_start(xt[:, :],
                                    xT_ap[:, 2 * off:2 * off + 2 * cap])
                xin[s] = xt
                tb = bcp.tile([128, cap], BF16, name=f"t1_{s}",
                              tag="t1", bufs=4)
                nc.gpsimd.dma_start(
                    tb[:, :], tp_ap[1:2, off:off + cap].partition_broadcast(128))
                t1s[s] = tb

            def vc_layer(s, L, xin_t, store):
                """layers 0/1: (o,b) = relu(C.T @ [x;z1] + Cb.T @ tps)"""
                cap, off = caps[s], offs[s]
                if (L, s) not in wts:
                    wload(L, s)
                wt = wts.pop((L, s))

                z1 = zp.tile([128, 2 * cap], BF16, name=f"z1_{L}_{s}",
                             tag="z1", bufs=3)
                for h in range(2):
                    nc.vector.tensor_mul(z1[:, h * cap:(h + 1) * cap],
                                         xin_t[:, h * cap:(h + 1) * cap],
                                         t1s[s][:, :])
                rhs_of = [xin_t, xin_t, z1, z1]
                # x k-tiles first: their operand lands well before z1 (which
                # needs the dt broadcast + DVE) in the prologue
                kt_order = (0, 1, 2, 3)
                xo = actp.tile([128, 2 * cap], BF16, name=f"x{L + 1}_{s}",
                               tag=f"xo{L}", bufs=3)
                for m in range(2):
                    ps = pmp.tile([128, cap], F32, name=f"pm{L}_{s}_{m}",
                                  tag="pm", bufs=6)
                    for j, kt in enumerate(kt_order):
                        h = kt % 2
                        nc.tensor.matmul(
                            ps[:, :],
                            wt[:, kt * D + m * 128:kt * D + (m + 1) * 128],
                            rhs_of[kt][:, h * cap:(h + 1) * cap],
                            start=(j == 0), stop=False)
                    nc.tensor.matmul(
                        ps[:, :],
                        cbw[0:NP, (L * nslot + s) * D + m * 128:
                            (L * nslot + s) * D + (m + 1) * 128],
                        tps[0:NP, off:off + cap],
                        start=False, stop=True)
                    nc.scalar.activation(xo[:, m * cap:(m + 1) * cap],
                                         ps[:, :], RELU)
                store[s] = xo

            def head_layer(s):
                """layer 2 (out_dim=1): q = C2.T @ x2; out = (q0+b0) + (q1+b1)*dt"""
                cap, off = caps[s], offs[s]
                psq = pqp.tile([NP, cap], F32, name=f"pq{s}", tag="pq", bufs=1)
                for h in range(2):
                    nc.tensor.matmul(psq[:, :],
                                     c2t[:, s * 2 * NP + h * NP:
                                         s * 2 * NP + (h + 1) * NP],
                                     x2[s][:, h * cap:(h + 1) * cap],
                                     start=(h == 0), stop=(h == 1))
                # fused (q + b) * tps in one DVE op
                rq = smp.tile([NP, cap], BF16, name=f"rq{s}", tag="rq", bufs=3)
                nc.vector.scalar_tensor_tensor(
                    rq[:, :], psq[:, :], c2b[0:NP, s:s + 1],
                    tps[0:NP, off:off + cap],
                    mybir.AluOpType.add, mybir.AluOpType.mult)
                psr = pqp.tile([1, cap], F32, name=f"pr{s}", tag="pr", bufs=1)
                nc.tensor.matmul(psr[:, :], ones2[:, :], rq[:, :],
                                 start=True, stop=True)
                nc.scalar.activation(out_all[0:1, off:off + cap],
                                     psr[:, :], COPY)
                nc.scalar.dma_start(out_ap[0:1, off:off + cap],
                                    out_all[0:1, off:off + cap])

            load_seg(0)
            load_seg(1)
            for step in range(nslot + 2):
                if step < nslot:
                    if step not in xin:
                        load_seg(step)
                    vc_layer(step, 0, xin[step], x1)
                    xin.pop(step)
                if 1 <= step < nslot + 1:
                    vc_layer(step - 1, 1, x1[step - 1], x2)
                    x1.pop(step - 1)
                if step >= 2:
                    head_layer(step - 2)
                    x2.pop(step - 2)
                    t1s.pop(step - 2)

    nc.compile()
    return nc


def _prep_host(treatment, features, W0, b0, W1, b1, W2, b2):
    t32 = np.asarray(treatment, dtype=np.float32)
    t = t32.astype(np.float64)
    x = np.asarray(features, dtype=np.float32)

    order = np.argsort(t32, kind='stable')
    percore = order.reshape(N_CORES, BPC)
    kn32 = KNOTS.astype(np.float32)

    chunk_lists = []                        # per core: list of index arrays
    for c in range(N_CORES):
        gi = percore[c]
        tc_ = t32[gi]
        bounds = {0, len(gi)}
        for k in kn32:
            bounds.add(int(np.searchsorted(tc_, k, side='right')))
        bounds = sorted(bounds)
        chunks = []
        for a, b_ in zip(bounds, bounds[1:]):
            r = b_ - a
            if r <= 0:
                continue
            k = -(-r // CAPMAX)
            base, rem = divmod(r, k)
            st = a
            for i in range(k):
                n = base + (1 if i < rem else 0)
                chunks.append(gi[st:st + n])
                st += n
        chunks.sort(key=len, reverse=True)
        chunk_lists.append(chunks)

    nslot = max(len(ch) for ch in chunk_lists)
    caps = tuple(max(8, -(-max(len(ch[i]) if i < len(ch) else 0
                               for ch in chunk_lists) // 8) * 8)
                 for i in range(nslot))
    assert max(caps) <= 512, caps
    offs = np.concatenate([[0], np.cumsum(caps)]).astype(np.int64)
    bp = int(offs[-1])

    g = _gamma4()
    c4s, cb4s = [], []
    for W, b in ((W0, b0), (W1, b1)):
        Ws = np.asarray(W, dtype=np.float64).reshape(SDIM, D, D)
        c4s.append(np.einsum('mps,sio->mpio', g, Ws))
        cb4s.append(np.einsum('mps,so->mpo', g, np.asarray(b, np.float64)))
    c4h = np.einsum('mps,si->mpi', g, np.asarray(W2, np.float64))
    cb4h = np.einsum('mps,s->mp', g, np.asarray(b2, np.float64)[:, 0])

    gather = np.full((N_CORES, bp), -1, dtype=np.int64)
    in_maps = []
    for c in range(N_CORES):
        chunks = chunk_lists[c]
        xT = np.zeros((128, 2 * bp), dtype=NPBF)
        tp = np.zeros((NP, bp), dtype=NPBF)
        cw = [np.zeros((nslot, 128, KT * D), dtype=NPBF) for _ in range(2)]
        cbw = np.zeros((NP, 2 * nslot * D), dtype=NPBF)
        c2 = np.zeros((128, nslot * 2 * NP), dtype=NPBF)
        c2b = np.zeros((NP, nslot), dtype=np.float32)
        for s, gi in enumerate(chunks):
            n, off = len(gi), int(offs[s])
            tv = t[gi]
            t0 = (tv.min() + tv.max()) / 2
            h = max((tv.max() - tv.min()) / 2, 1e-9)
            m = int(np.searchsorted(kn32, t32[gi[0]], side='right'))
            gather[c, off:off + n] = gi
            cap_s = caps[s]
            xT[:, 2 * off:2 * off + n] = x[gi, 0:128].T.astype(NPBF)
            xT[:, 2 * off + cap_s:2 * off + cap_s + n] = \
                x[gi, 128:256].T.astype(NPBF)
            dv = tv - t0
            tp[:, off:off + n] = np.stack([np.ones_like(dv), dv]).astype(NPBF)
            for L in range(2):
                c2l = _relin(c4s[L][m], t0, h)              # (2, 256, 256)
                c3r = c2l.reshape(NP * D, D)
                for kt in range(KT):
                    cw[L][s, :, kt * D:(kt + 1) * D] = \
                        c3r[kt * 128:(kt + 1) * 128, :].astype(NPBF)
                cbl = _relin(cb4s[L][m], t0, h)             # (2, 256)
                cbw[:, (L * nslot + s) * D:(L * nslot + s + 1) * D] = \
                    cbl.astype(NPBF)
            c2h = _relin(c4h[m], t0, h)                     # (2, 256)
            for hh in range(2):
                c2[:, s * 2 * NP + hh * NP:s * 2 * NP + (hh + 1) * NP] = \
                    c2h[:, hh * 128:(hh + 1) * 128].T.astype(NPBF)
            c2b[:, s] = _relin(cb4h[m][:, None], t0, h)[:, 0].astype(np.float32)
        in_maps.append(dict(
            xT=np.ascontiguousarray(xT), tp=np.ascontiguousarray(tp),
            c0w=np.ascontiguousarray(cw[0]), c1w=np.ascontiguousarray(cw[1]),
            cbw=cbw, c2=c2, c2b=c2b, ones2=np.ones((NP, 1), NPBF)))
    return caps, in_maps, gather


def kernel(treatment, features, W0, b0, W1, b1, W2, b2):
    global LAST_EXEC_NS, LAST_MEAN_EXEC_NS, LAST_RES
    caps, in_maps, gather = _prep_host(treatment, features, W0, b0, W1, b1,
                                       W2, b2)

    if caps not in _PROG_CACHE:
        _PROG_CACHE[caps] = _build_program(caps)
    nc = _PROG_CACHE[caps]

    if TRACE:
        _register_ntff_hook()
    res = bass_utils.run_bass_kernel_spmd(
        nc, in_maps, core_ids=list(range(N_CORES)), trace=TRACE)
    LAST_EXEC_NS = res.exec_time_ns
    LAST_MEAN_EXEC_NS = res.mean_exec_time_ns
    LAST_RES = res

    out = np.empty((B,), dtype=np.float32)
    for c in range(N_CORES):
        row = res.results[c]["out"][0]
        v = gather[c] >= 0
        out[gather[c][v]] = row[v]
    return out.reshape(B, 1)
